# revision 73
# baseline (speedup 1.0000x reference)
"""Causal self-attention with RoPE on 8 trn2 NeuronCores.

Problem: B=2, T=2048, D=1024, H=16 heads, head_dim=64, fp32.
Sharding: core = b*4 + g  (data parallel over batch, tensor parallel over
head groups of 4). Each core computes its 4 heads' attention plus the
row-slice of the output projection; the host sums the 4 partial Y^T per
batch and transposes back.

v3 (181.6us -> 140.3us): bf16 end-to-end (half the DMA bytes, full-rate
matmuls at every tile size, 2x DVE on SBUF elementwise), hh-paired score
tiles [128,2,512] across two PSUM banks so one exp covers both head halves,
all PSUM->SBUF staging on DVE/ACT (GPSIMD cannot touch PSUM), softmax
normalization via gpsimd partition_broadcast (no PSUM-PSUM tensor ops),
paired output stores (one DMA per two blocks), C(i-1)/A(i+1) units
interleaved into B(i)'s emission slots, a PE p-state warmup burst under the
initial DMA window, and A(0)'s accs/vaccs borrowing the then-idle oacc
banks so nothing waits on the sin-table DMA.

Per-core dataflow (everything transposed so matmuls contract on partitions):
  xT (1024, 2048)  =  x[b].T                     [ExternalInput, bf16]
  QT/KT packs [128, 2048] (2 heads of 64 rows)   = Wq/Wk-slices^T @ xT
  RoPE: QT' = QT*cos + (R2 @ QT)*sin   (R2 = block-diag rotate-half matrix)
  V_aug [128, 16, 260]: V natural layout per key block, 4 heads x (64 dims
      + ones column) -> fused softmax denominator.
  S^T tile [keys 128, q 512] = KT'_h-slice^T @ QT'_h  (PE, K=64)
  P^T = exp(S^T * 0.125) (ACT), diag blocks masked by DVE mul.
  Oacc [65, 512] += V_aug_h-slice^T @ P^T  (row 64 = denominator)
  O^T = Oacc[0:64] * bcast(1/denom)   (PE ones-matmul broadcast + DVE mul)
  Y^T partial [1024, 2048] = Wp-slice^T @ O^T packs  -> DRAM out.
"""

import sys
import numpy as np

sys.path.insert(0, "/opt/trn_rl_repo")

import ml_dtypes

BF16 = ml_dtypes.bfloat16

B, T, D, H = 2, 2048, 1024, 16
HD = 64          # head dim
HPC = 4          # heads per core
NCORES = 8
ROPE_BASE = 10000.0

_PROGRAM = None  # cached compiled program


def _rope_tables_np():
    inv_freq = 1.0 / (ROPE_BASE ** (np.arange(0, HD, 2, dtype=np.float32) / np.float32(HD)))
    pos = np.arange(T, dtype=np.float32)
    freqs = np.outer(pos, inv_freq).astype(np.float32)          # (T, 32)
    emb = np.concatenate([freqs, freqs], axis=-1)               # (T, 64)
    cosT = np.cos(emb).T.astype(np.float32)                     # (64, T)
    sinT = np.sin(emb).T.astype(np.float32)
    cos2 = np.vstack([cosT, cosT]).copy()                       # (128, T) two heads
    sin2 = np.vstack([sinT, sinT]).copy()
    return cos2, sin2


def _r2_np():
    # qrot[d] = -q[d+32] (d<32) ; q[d-32] (d>=32), per 64-row block.
    # matmul computes out[d, t] = sum_k r2[k, d] q[k, t]
    r2 = np.zeros((128, 128), dtype=np.float32)
    for base in (0, 64):
        for d in range(32):
            r2[base + d + 32, base + d] = -1.0
            r2[base + d, base + d + 32] = 1.0
    return r2


def _masks_np():
    # tri[j, ql] = 1 if key j may attend query ql within a diagonal block
    j = np.arange(128)[:, None]
    ql = np.arange(128)[None, :]
    return (j <= ql).astype(np.float32)                         # [128, 128]


def build_program():
    import concourse.bass as bass
    import concourse.tile as tile
    from concourse import bacc, mybir
    from contextlib import ExitStack

    f32 = mybir.dt.float32
    bf16 = mybir.dt.bfloat16
    fmm = bf16

    nc = bacc.Bacc(None, target_bir_lowering=False, debug=False)

    # xT pre-tiled on host: xTr[kc, tch, p, t] = x[b].T[kc*128+p, tch*512+t]
    # so each [128, 512] working tile is a contiguous 128 KB DMA.
    xT = nc.declare_dram_parameter("xT", [D // 128, T // 512, 128, 512], fmm, isOutput=False)
    wq = nc.declare_dram_parameter("wq", [D, 256], fmm, isOutput=False)
    wk = nc.declare_dram_parameter("wk", [D, 256], fmm, isOutput=False)
    wv = nc.declare_dram_parameter("wv", [D, 256], fmm, isOutput=False)
    wp = nc.declare_dram_parameter("wp", [256, D], fmm, isOutput=False)
    # yT tiled the same way: yTr[ech, tch, p, t] = yT_partial[ech*128+p, tch*512+t]
    yT = nc.declare_dram_parameter("yT", [8, T // 512, 128, 512], bf16, isOutput=True)

    cos2_np, sin2_np = _rope_tables_np()
    cos_d = nc.inline_tensor(cos2_np[:64].astype(BF16), name="cos2")
    sin_d = nc.inline_tensor(sin2_np[:64].astype(BF16), name="sin2")
    r2_d = nc.inline_tensor(_r2_np().astype(BF16), name="r2")
    masks_d = nc.inline_tensor(_masks_np().astype(BF16), name="masks")

    NT = T // 512            # 4 t-chunks
    NJ = T // 128            # 16 key blocks
    KC = D // 128            # 8 contraction chunks

    with tile.TileContext(nc) as tc, ExitStack() as ctx:
        # --- persistent SBUF ---
        wts = ctx.enter_context(tc.tile_pool(name="wts", bufs=1))
        packs = ctx.enter_context(tc.tile_pool(name="packs", bufs=1))
        consts = ctx.enter_context(tc.tile_pool(name="consts", bufs=1))

        # --- working pools (xts first: its loads gate the first matmuls) ---
        xts = ctx.enter_context(tc.tile_pool(name="xts", bufs=2))

        wq_sb = wts.tile([128, KC, 256], fmm, tag="wq")
        wk_sb = wts.tile([128, KC, 256], fmm, tag="wk")
        wv_sb = wts.tile([128, KC, 256], fmm, tag="wv")
        wp_sb = wts.tile([128, 2, 1024], fmm, tag="wp")
        xt_tiles = {}

        xt_done = {}

        def prefetch_xt(tch, s_lo=0, s_hi=None):
            nsplit = 4 if tch == 0 else 2
            if s_hi is None:
                s_hi = nsplit
            if tch >= NT:
                return
            if tch in xt_tiles:
                xt = xt_tiles[tch]
            else:
                xt = xts.tile([128, KC, 512], fmm, tag="xt", name=f"xt{tch}")
                xt_tiles[tch] = xt
            step = KC // nsplit
            for s in range(s_lo, s_hi):
                if (tch, s) in xt_done:
                    continue
                xt_done[(tch, s)] = True
                nc.sync.dma_start(
                    out=xt[:, s * step:(s + 1) * step, :],
                    in_=xT[s * step:(s + 1) * step, tch].rearrange("k p t -> p k t"))

        cos_sb = consts.tile([128, T], bf16, tag="cos")
        sin_sb = consts.tile([128, T], bf16, tag="sin")
        r2_sb = consts.tile([128, 128], bf16, tag="r2")
        masks_sb = consts.tile([128, 128], bf16, tag="masks")

        # PE p-state warmup: the cost model ramps 0.65 -> 1.2 -> 2.4 GHz over
        # ~3us of continuous execution. Burn the ramp on dummy matmuls into a
        # scratch PSUM bank while the first weight/x DMAs are in flight.
        # Memset on Pool so the warmup isn't gated on DVE startup.
        warm_src = consts.tile([128, 256], fmm, tag="warm")
        nc.gpsimd.memset(warm_src[:], 0.0)

        # serial-DMA issue order == need order: wq+xt0 gate the first matmuls,
        # then wk (K proj), cos/sin/r2 (rope), wv (V proj), masks, wp (C).
        # cos/sin halves are identical across the two head rows: load 64
        # partitions from DRAM and duplicate on-chip via SBUF->SBUF DMA.
        wq_r = wq.rearrange("(k p) d -> p k d", p=128)
        wk_r = wk.rearrange("(k p) d -> p k d", p=128)
        nc.sync.dma_start(out=wq_sb[:, 0:4, :], in_=wq_r[:, 0:4, :])
        prefetch_xt(0, 0, 2)
        nc.sync.dma_start(out=wq_sb[:, 4:8, :], in_=wq_r[:, 4:8, :])
        prefetch_xt(0, 2, 3)
        nc.sync.dma_start(out=wk_sb[:, 0:4, :], in_=wk_r[:, 0:4, :])
        prefetch_xt(0, 3, 4)
        nc.sync.dma_start(out=wk_sb[:, 4:8, :], in_=wk_r[:, 4:8, :])
        nc.sync.dma_start(out=r2_sb[:], in_=r2_d[:, :])
        nc.sync.dma_start(out=cos_sb[0:64, :], in_=cos_d[:, :])
        nc.sync.dma_start(out=sin_sb[0:64, :], in_=sin_d[:, :])
        nc.sync.dma_start(out=cos_sb[64:128, :], in_=cos_sb[0:64, :])
        nc.sync.dma_start(out=sin_sb[64:128, :], in_=sin_sb[0:64, :])
        nc.sync.dma_start(out=wv_sb[:], in_=wv.rearrange("(k p) d -> p k d", p=128))
        nc.sync.dma_start(out=masks_sb[:], in_=masks_d[:, :])
        nc.sync.dma_start(out=wp_sb[:], in_=wp.rearrange("(k p) d -> p k d", p=128))

        onescol = consts.tile([128, NJ, HPC], fmm, tag="onescol")
        nc.vector.memset(onescol[:], 1.0)
        ones64 = consts.tile([1, 64], fmm, tag="ones64")
        nc.vector.memset(ones64[:], 1.0)

        qt_sb = [packs.tile([128, T], fmm, tag=f"qt{p}", name=f"qt{p}") for p in range(2)]
        kt_sb = [packs.tile([128, T], fmm, tag=f"kt{p}", name=f"kt{p}") for p in range(2)]
        ot_sb = [packs.tile([128, T], fmm, tag=f"ot{p}", name=f"ot{p}") for p in range(2)]
        v_sb = packs.tile([128, NJ, HPC * (HD + 1)], fmm, tag="vaug")

        # ones columns of v_aug (fused softmax denominator)
        nc.vector.tensor_copy(
            v_sb[:].rearrange("p j (h c) -> p j h c", h=HPC)[:, :, :, HD:HD + 1],
            onescol[:],
        )

        # --- working pools ---
        tmps = ctx.enter_context(tc.tile_pool(name="tmps", bufs=3))
        pts = ctx.enter_context(tc.tile_pool(name="pts", bufs=16))
        outs = ctx.enter_context(tc.tile_pool(name="outs", bufs=8))
        smalls = ctx.enter_context(tc.tile_pool(name="smalls", bufs=2))

        # 8 PSUM banks: st2(2x2) double-buffered hh-paired score tiles,
        # aux(2) rotated by A's acc/rot/vacc + C's pc, o(2) for oacc pairs.
        psST = ctx.enter_context(tc.tile_pool(name="psST", bufs=2, space="PSUM"))
        psAUX = ctx.enter_context(tc.tile_pool(name="psAUX", bufs=2, space="PSUM"))
        psO = ctx.enter_context(tc.tile_pool(name="psO", bufs=2, space="PSUM"))

        # p-state warmup burst (independent of all real data; runs at t~0).
        # Lives in the st2 pool, whose slots are first needed much later.
        warm_ps = psST.tile([128, 2, 512], f32, tag="st2", name="warm_ps")

        def warm(n):
            for _ in range(n):
                nc.tensor.matmul(warm_ps[:, 0, 0:256], warm_src[:, 0:128],
                                 warm_src[:], start=True, stop=True)
        warm(10)

        def emit_rope(tch, acc, raw, dst, p):
            ts = slice(tch * 512, (tch + 1) * 512)
            rot = psAUX.tile([128, 512], f32, tag="aux", name="rot")
            nc.tensor.matmul(rot[:], r2_sb[:], raw[:], start=True, stop=True)
            # tc_t: both operands bf16 SBUF -> DVE fast mode
            tc_t = tmps.tile([128, 512], bf16, tag="tc", name="tc_t")
            nc.vector.tensor_mul(tc_t[:], raw[:], cos_sb[:, ts])
            # ts_t: reads rot from PSUM (f32) -> 1x
            ts_t = tmps.tile([128, 512], bf16, tag="ts", name="ts_t")
            nc.vector.tensor_mul(ts_t[:], rot[:], sin_sb[:, ts])
            nc.vector.tensor_add(dst[p][:, ts], tc_t[:], ts_t[:])

        def attn_stream(qi, p, oaccs, pending, kj_lo, kj_hi, kj_max):
            """st2 (both head-halves) -> one exp -> (mask) -> O matmuls for
            kj in [kj_lo, kj_hi), software-pipelined so the PE never queues
            behind its own exp."""
            def emit_o(kj, pt2, lo):
                for hh in range(2):
                    hl = 2 * p + hh
                    nc.tensor.matmul(
                        oaccs[hh][0:65, lo:512],
                        v_sb[:, kj, 65 * hl:65 * hl + 65],
                        pt2[:, hh, lo:512],
                        start=(kj == 0), stop=(kj == kj_max - 1),
                    )

            for kj in range(kj_lo, kj_hi):
                r = kj - 4 * qi          # >= 0 on the causal diagonal
                lo = max(r, 0) * 128     # first valid column
                # drain ahead of the new st pair: the queued PV matmuls run
                # while the pair waits for its PSUM slot (exp of pair k-2)
                while len(pending) > 12:
                    emit_o(*pending.pop(0))
                st2 = psST.tile([128, 2, 512], f32, tag="st2", name="st2")
                for hh in range(2):
                    off = 64 * hh
                    nc.tensor.matmul(
                        st2[:, hh, lo:512],
                        kt_sb[p][off:off + 64, kj * 128:(kj + 1) * 128],
                        qt_sb[p][off:off + 64, qi * 512 + lo:(qi + 1) * 512],
                        start=True, stop=True,
                    )
                pt2 = pts.tile([128, 2, 512], fmm, tag="pt", name="pt2")
                nc.scalar.activation(
                    pt2[:, :, lo:512], st2[:, :, lo:512],
                    mybir.ActivationFunctionType.Exp, scale=0.125,
                )
                if r >= 0:
                    for hh in range(2):
                        nc.vector.tensor_mul(
                            pt2[:, hh, lo:lo + 128], pt2[:, hh, lo:lo + 128],
                            masks_sb[:],
                        )
                pending.append((kj, pt2, lo))
            if kj_hi == kj_max:
                while pending:
                    emit_o(*pending.pop(0))

        def attn_norm(qi, p, oaccs):
            qs = slice(qi * 512, (qi + 1) * 512)
            final = qi == NT - 1 and p == 1
            for hh in range(2):
                off = 64 * hh
                recip = smalls.tile([1, 512], fmm, tag="recip", name="recip")
                with nc.allow_low_precision(reason="bf16 rounding of softmax recip"):
                    nc.vector.reciprocal(recip[:], oaccs[hh][64:65, :])
                bc = smalls.tile([64, 512], fmm, tag="bc", name="bc")
                if final:
                    # latency-critical last stream: PE broadcast + ACT copies
                    # (both engines idle here), bf16 operands so the closing
                    # DVE multiply runs in 2x mode.
                    bc_ps = psST.tile([128, 2, 512], f32, tag="st2", name="bc_ps")
                    nc.tensor.matmul(bc_ps[0:64, 0, :], ones64[:], recip[:],
                                     start=True, stop=True)
                    nc.scalar.copy(bc[:], bc_ps[0:64, 0, :])
                    osb = smalls.tile([64, 512], fmm, tag="osb", name="osb")
                    nc.scalar.copy(osb[:], oaccs[hh][0:64, :])
                    nc.vector.tensor_mul(ot_sb[p][off:off + 64, qs], osb[:], bc[:])
                else:
                    nc.gpsimd.partition_broadcast(bc[:], recip[:])
                    nc.vector.tensor_mul(ot_sb[p][off:off + 64, qs], oaccs[hh][0:64, :], bc[:])

        def a_unit_list(tch):
            """A(tch) as a list of emission closures (proj groups, V blocks).
            The rope skew chains across units via `state`."""
            if tch >= NT:
                return []
            state = {"pend": None}

            def start():
                prefetch_xt(tch)
                prefetch_xt(tch + 1)

            def qk_group(w_sb, dst, p, chain_warm=False):
                def emit():
                    xt = xt_tiles[tch]
                    # A(0) runs before B(0): the oacc banks are free, so acc
                    # and vacc chains use them, keeping the 2 aux slots for
                    # rot tiles only (a rot between two accs would make vacc
                    # wait on the sin-DMA-gated rope multiply).
                    if tch == 0:
                        acc = psO.tile([128, 512], f32, tag="oacc", name=f"acc{tch}_{p}")
                    else:
                        acc = psAUX.tile([128, 512], f32, tag="aux", name=f"acc{tch}_{p}")
                    for kc in range(KC):
                        nc.tensor.matmul(
                            acc[:],
                            w_sb[:, kc, 128 * p:128 * (p + 1)],
                            xt[:, kc, :],
                            start=(kc == 0), stop=(kc == KC - 1),
                        )
                        if chain_warm and kc in (1, 3, 5):
                            warm(3)  # keep PE busy while DMA feeds the chain
                    raw = tmps.tile([128, 512], fmm, tag="raw", name="raw")
                    nc.vector.tensor_copy(raw[:], acc[:])
                    if state["pend"] is not None:
                        emit_rope(*state["pend"])
                    state["pend"] = (tch, acc, raw, dst, p)
                return emit

            def v_block(jb):
                def emit():
                    xt = xt_tiles[tch]
                    if state["pend"] is not None:
                        emit_rope(*state["pend"])
                        state["pend"] = None
                    jbg = tch * 4 + jb
                    if tch == 0:
                        vacc = psO.tile([128, 256], f32, tag="oacc", name=f"vacc{jbg}")
                    else:
                        vacc = psAUX.tile([128, 256], f32, tag="aux", name=f"vacc{jbg}")
                    for kc in range(KC):
                        nc.tensor.matmul(
                            vacc[:],
                            xt[:, kc, 128 * jb:128 * (jb + 1)],
                            wv_sb[:, kc, :],
                            start=(kc == 0), stop=(kc == KC - 1),
                        )
                    nc.vector.tensor_copy(
                        v_sb[:].rearrange("p j (h c) -> p j h c", h=HPC)[:, jbg, :, 0:HD],
                        vacc[:].rearrange("p (h c) -> p h c", h=HPC),
                    )
                return emit

            units = [start]
            units.append(qk_group(wq_sb, qt_sb, 0, chain_warm=(tch == 0)))
            if tch == 0:
                pass
                # keep the PE busy-streak continuous while DMA feeds the
                # first chains, so the p-state ramp completes early; pull the
                # wv-gated V blocks ahead of the pack-1 chains (p0's stream
                # needs them first anyway)
                units.append(lambda: warm(3))
                units.append(qk_group(wk_sb, kt_sb, 0))
                units.append(lambda: warm(3))
                units.append(v_block(0))
                units.append(v_block(1))
                units.append(qk_group(wq_sb, qt_sb, 1))
                units.append(qk_group(wk_sb, kt_sb, 1))
                units.append(v_block(2))
                units.append(v_block(3))
            else:
                units.append(qk_group(wk_sb, kt_sb, 0))
                units.append(v_block(0))
                units.append(v_block(1))
                units.append(qk_group(wq_sb, qt_sb, 1))
                units.append(qk_group(wk_sb, kt_sb, 1))
                units.append(v_block(2))
                units.append(v_block(3))
            units.append(lambda: xt_tiles.pop(tch, None))
            return units

        def c_unit_list(tch):
            ts = slice(tch * 512, (tch + 1) * 512)
            last = tch == NT - 1
            oc2s = {}

            def c_block(ech):
                def emit():
                    # The trailing C(3) has no B to hide in: alternate its pc
                    # tiles over two pools (4 banks) and spread the PSUM->SBUF
                    # copies over three engines so the pc WAR chain pipelines.
                    if last and ech % 2 == 1:
                        pc = psO.tile([128, 512], f32, tag="oacc", name=f"pc{tch}_{ech}")
                    else:
                        pc = psAUX.tile([128, 512], f32, tag="aux", name=f"pc{tch}_{ech}")
                    for kd in range(2):
                        nc.tensor.matmul(
                            pc[:],
                            wp_sb[:, kd, ech * 128:(ech + 1) * 128],
                            ot_sb[kd][:, ts],
                            start=(kd == 0), stop=(kd == 1),
                        )
                    # pair two ech blocks into one [128, 2, 512] staging tile
                    # -> a single DMA per pair (halves the per-store HWDGE tax)
                    pair = ech // 2
                    if pair not in oc2s:
                        oc2s[pair] = outs.tile([128, 2, 512], bf16, tag="oc", name="oc2")
                    oc2 = oc2s[pair]
                    half = ech % 2
                    # GPSIMD cannot read PSUM: staging copies on DVE, with ACT
                    # only for the odd blocks of the exposed trailing C
                    if last and ech % 2 == 1:
                        nc.scalar.copy(oc2[:, half, :], pc[:])
                    else:
                        nc.vector.tensor_copy(oc2[:, half, :], pc[:])
                    if half == 1:
                        nc.sync.dma_start(
                            out=yT[2 * pair:2 * pair + 2, tch].rearrange("e p t -> p e t"),
                            in_=oc2[:])
                return emit
            return [c_block(e) for e in range(8)]

        def b_emit(qi, fill_units):
            """Emit B(qi)'s two attention streams, sprinkling `fill_units`
            (A(qi+1) / C(qi-1) closures) between kj iterations so the PE
            always has independent matmul work queued behind exp waits."""
            kj_max = 4 * (qi + 1)
            # for the last qi, all fill must land during p=0: p=1's oaccs come
            # from the aux pool, and fill C-blocks allocated after them would
            # chain behind the whole p=1 stream.
            n_slots = (kj_max + 1) if qi == NT - 1 else (2 * kj_max + 2)
            fill = list(fill_units)
            n_fill = len(fill)
            slot = [0]

            def maybe_fill():
                # spread the n_fill units evenly across the n_slots slots
                k = (n_fill * (slot[0] + 1)) // n_slots - (n_fill * slot[0]) // n_slots
                for _ in range(k):
                    if fill:
                        fill.pop(0)()
                slot[0] += 1

            for p in range(2):
                # qi=3 p=1: take oaccs from the aux pool so this stream's
                # emit_o doesn't wait on p=0's norm to release the psO slots
                # (there is no A/C fill left to hide that wait behind).
                if qi == NT - 1 and p == 1:
                    pool, tag = psAUX, "aux"
                else:
                    pool, tag = psO, "oacc"
                oaccs = [pool.tile([128, 512], f32, tag=tag, name=f"oacc{qi}_{p}_{j}")
                         for j in range(2)]
                pending = []
                for kj in range(kj_max):
                    attn_stream(qi, p, oaccs, pending, kj, kj + 1, kj_max)
                    if p == 0 or qi != NT - 1:
                        maybe_fill()
                attn_norm(qi, p, oaccs)
                if p == 0 or qi != NT - 1:
                    maybe_fill()
                if p == 0 and qi == NT - 1:
                    while fill:
                        fill.pop(0)()
            while fill:
                fill.pop(0)()

        # software pipeline: A(0); B(0) with A(1) filled in; B(i) with
        # C(i-1)+A(i+1) filled in; C(1) deferred to B(3), which is otherwise
        # fill-starved (no A(4)); trailing C(3).
        for u in a_unit_list(0):
            u()
        b_emit(0, a_unit_list(1))
        b_emit(1, c_unit_list(0) + a_unit_list(2))
        b_emit(2, a_unit_list(3))
        b_emit(3, c_unit_list(1) + c_unit_list(2))
        for u in c_unit_list(3):
            u()

    nc.compile()
    return nc


def get_program():
    global _PROGRAM
    if _PROGRAM is None:
        _PROGRAM = build_program()
    return _PROGRAM


def make_in_maps(x, W_qkv, W_proj):
    x = np.asarray(x, dtype=np.float32)
    W_qkv = np.asarray(W_qkv, dtype=np.float32)
    W_proj = np.asarray(W_proj, dtype=np.float32)
    in_maps = []
    xtr = {}
    for b in range(B):
        xt = x[b].T.reshape(D // 128, 128, T // 512, 512)
        xtr[b] = np.ascontiguousarray(xt.transpose(0, 2, 1, 3)).astype(BF16)
    for core in range(NCORES):
        b, g = divmod(core, 4)
        cs = slice(g * 256, (g + 1) * 256)
        in_maps.append({
            "xT": xtr[b],
            "wq": np.ascontiguousarray(W_qkv[:, 0 * D:1 * D][:, cs]).astype(BF16),
            "wk": np.ascontiguousarray(W_qkv[:, 1 * D:2 * D][:, cs]).astype(BF16),
            "wv": np.ascontiguousarray(W_qkv[:, 2 * D:3 * D][:, cs]).astype(BF16),
            "wp": np.ascontiguousarray(W_proj[cs, :]).astype(BF16),
        })
    return in_maps


def gather_output(results):
    out = np.empty((B, T, D), dtype=np.float32)
    for b in range(B):
        acc = results[4 * b]["yT"].astype(np.float32)
        for g in range(1, 4):
            acc += results[4 * b + g]["yT"].astype(np.float32)
        # (ech, tch, p, t) -> yT (D, T) -> transpose to (T, D)
        yt = acc.transpose(0, 2, 1, 3).reshape(D, T)
        out[b] = yt.T
    return out


def kernel(x, W_qkv, W_proj, key_padding_mask=None, **_ignored):
    # key_padding_mask is all-True per the problem spec (fill: ones) -> no-op.
    from concourse.bass_utils import run_bass_kernel_spmd

    nc = get_program()
    in_maps = make_in_maps(x, W_qkv, W_proj)
    res = run_bass_kernel_spmd(nc, in_maps, list(range(NCORES)))
    return gather_output(res.results)


# revision 81
# speedup vs baseline: 1.0203x; 1.0203x over previous
"""Causal self-attention with RoPE on 8 trn2 NeuronCores.

Problem: B=2, T=2048, D=1024, H=16 heads, head_dim=64, fp32.
Sharding: core = b*4 + g  (data parallel over batch, tensor parallel over
head groups of 4). Each core computes its 4 heads' attention plus the
row-slice of the output projection; the host sums the 4 partial Y^T per
batch and transposes back.

v3 (181.6us -> 140.3us): bf16 end-to-end (half the DMA bytes, full-rate
matmuls at every tile size, 2x DVE on SBUF elementwise), hh-paired score
tiles [128,2,512] across two PSUM banks so one exp covers both head halves,
all PSUM->SBUF staging on DVE/ACT (GPSIMD cannot touch PSUM), softmax
normalization via gpsimd partition_broadcast (no PSUM-PSUM tensor ops),
paired output stores (one DMA per two blocks), C(i-1)/A(i+1) units
interleaved into B(i)'s emission slots, a PE p-state warmup burst under the
initial DMA window, and A(0)'s accs/vaccs borrowing the then-idle oacc
banks so nothing waits on the sin-table DMA.

Per-core dataflow (everything transposed so matmuls contract on partitions):
  xT (1024, 2048)  =  x[b].T                     [ExternalInput, bf16]
  QT/KT packs [128, 2048] (2 heads of 64 rows)   = Wq/Wk-slices^T @ xT
  RoPE: QT' = QT*cos + (R2 @ QT)*sin   (R2 = block-diag rotate-half matrix)
  V_aug [128, 16, 260]: V natural layout per key block, 4 heads x (64 dims
      + ones column) -> fused softmax denominator.
  S^T tile [keys 128, q 512] = KT'_h-slice^T @ QT'_h  (PE, K=64)
  P^T = exp(S^T * 0.125) (ACT), diag blocks masked by DVE mul.
  Oacc [65, 512] += V_aug_h-slice^T @ P^T  (row 64 = denominator)
  O^T = Oacc[0:64] * bcast(1/denom)   (PE ones-matmul broadcast + DVE mul)
  Y^T partial [1024, 2048] = Wp-slice^T @ O^T packs  -> DRAM out.
"""

import sys
import numpy as np

sys.path.insert(0, "/opt/trn_rl_repo")

import ml_dtypes

BF16 = ml_dtypes.bfloat16

B, T, D, H = 2, 2048, 1024, 16
HD = 64          # head dim
HPC = 4          # heads per core
NCORES = 8
ROPE_BASE = 10000.0

_PROGRAM = None  # cached compiled program


def _rope_tables_np():
    inv_freq = 1.0 / (ROPE_BASE ** (np.arange(0, HD, 2, dtype=np.float32) / np.float32(HD)))
    pos = np.arange(T, dtype=np.float32)
    freqs = np.outer(pos, inv_freq).astype(np.float32)          # (T, 32)
    emb = np.concatenate([freqs, freqs], axis=-1)               # (T, 64)
    cosT = np.cos(emb).T.astype(np.float32)                     # (64, T)
    sinT = np.sin(emb).T.astype(np.float32)
    cos2 = np.vstack([cosT, cosT]).copy()                       # (128, T) two heads
    sin2 = np.vstack([sinT, sinT]).copy()
    return cos2, sin2


def _r2_np():
    # qrot[d] = -q[d+32] (d<32) ; q[d-32] (d>=32), per 64-row block.
    # matmul computes out[d, t] = sum_k r2[k, d] q[k, t]
    r2 = np.zeros((128, 128), dtype=np.float32)
    for base in (0, 64):
        for d in range(32):
            r2[base + d + 32, base + d] = -1.0
            r2[base + d, base + d + 32] = 1.0
    return r2


def _masks_np():
    # tri[j, ql] = 1 if key j may attend query ql within a diagonal block
    j = np.arange(128)[:, None]
    ql = np.arange(128)[None, :]
    return (j <= ql).astype(np.float32)                         # [128, 128]


def build_program():
    import concourse.bass as bass
    import concourse.tile as tile
    from concourse import bacc, mybir
    from contextlib import ExitStack

    f32 = mybir.dt.float32
    bf16 = mybir.dt.bfloat16
    fmm = bf16

    nc = bacc.Bacc(None, target_bir_lowering=False, debug=False)

    # xT pre-tiled on host: xTr[kc, tch, p, t] = x[b].T[kc*128+p, tch*512+t]
    # so each [128, 512] working tile is a contiguous 128 KB DMA.
    xT = nc.declare_dram_parameter("xT", [D // 128, T // 512, 128, 512], fmm, isOutput=False)
    wq = nc.declare_dram_parameter("wq", [D, 256], fmm, isOutput=False)
    wk = nc.declare_dram_parameter("wk", [D, 256], fmm, isOutput=False)
    wv = nc.declare_dram_parameter("wv", [D, 256], fmm, isOutput=False)
    wp = nc.declare_dram_parameter("wp", [256, D], fmm, isOutput=False)
    # yT tiled the same way: yTr[ech, tch, p, t] = yT_partial[ech*128+p, tch*512+t]
    yT = nc.declare_dram_parameter("yT", [8, T // 512, 128, 512], bf16, isOutput=True)

    cos2_np, sin2_np = _rope_tables_np()
    cos_d = nc.inline_tensor(cos2_np[:64].astype(BF16), name="cos2")
    sin_d = nc.inline_tensor(sin2_np[:64].astype(BF16), name="sin2")
    r2_d = nc.inline_tensor(_r2_np().astype(BF16), name="r2")
    masks_d = nc.inline_tensor(_masks_np().astype(BF16), name="masks")

    NT = T // 512            # 4 t-chunks
    NJ = T // 128            # 16 key blocks
    KC = D // 128            # 8 contraction chunks

    with tile.TileContext(nc) as tc, ExitStack() as ctx:
        # --- persistent SBUF ---
        wts = ctx.enter_context(tc.tile_pool(name="wts", bufs=1))
        packs = ctx.enter_context(tc.tile_pool(name="packs", bufs=1))
        consts = ctx.enter_context(tc.tile_pool(name="consts", bufs=1))

        # --- working pools (xts first: its loads gate the first matmuls) ---
        xts = ctx.enter_context(tc.tile_pool(name="xts", bufs=2))

        wq_sb = wts.tile([128, KC, 256], fmm, tag="wq")
        wk_sb = wts.tile([128, KC, 256], fmm, tag="wk")
        wv_sb = wts.tile([128, KC, 256], fmm, tag="wv")
        wp_sb = wts.tile([128, 2, 1024], fmm, tag="wp")
        xt_tiles = {}

        xt_done = {}

        def prefetch_xt(tch, s_lo=0, s_hi=None):
            nsplit = 4 if tch == 0 else 2
            if s_hi is None:
                s_hi = nsplit
            if tch >= NT:
                return
            if tch in xt_tiles:
                xt = xt_tiles[tch]
            else:
                xt = xts.tile([128, KC, 512], fmm, tag="xt", name=f"xt{tch}")
                xt_tiles[tch] = xt
            step = KC // nsplit
            for s in range(s_lo, s_hi):
                if (tch, s) in xt_done:
                    continue
                xt_done[(tch, s)] = True
                nc.sync.dma_start(
                    out=xt[:, s * step:(s + 1) * step, :],
                    in_=xT[s * step:(s + 1) * step, tch].rearrange("k p t -> p k t"))

        cos_sb = consts.tile([128, T], bf16, tag="cos")
        sin_sb = consts.tile([128, T], bf16, tag="sin")
        r2_sb = consts.tile([128, 128], bf16, tag="r2")
        masks_sb = consts.tile([128, 128], bf16, tag="masks")

        # PE p-state warmup: the cost model ramps 0.65 -> 1.2 -> 2.4 GHz over
        # ~3us of continuous execution. Burn the ramp on dummy matmuls into a
        # scratch PSUM bank while the first weight/x DMAs are in flight.
        # Memset on Pool so the warmup isn't gated on DVE startup.
        warm_src = consts.tile([128, 256], fmm, tag="warm")
        nc.gpsimd.memset(warm_src[:], 0.0)

        # serial-DMA issue order == need order: wq+xt0 gate the first matmuls,
        # then wk (K proj), cos/sin/r2 (rope), wv (V proj), masks, wp (C).
        # cos/sin halves are identical across the two head rows: load 64
        # partitions from DRAM and duplicate on-chip via SBUF->SBUF DMA.
        wq_r = wq.rearrange("(k p) d -> p k d", p=128)
        wk_r = wk.rearrange("(k p) d -> p k d", p=128)
        nc.sync.dma_start(out=wq_sb[:, 0:4, :], in_=wq_r[:, 0:4, :])
        prefetch_xt(0, 0, 2)
        nc.sync.dma_start(out=wq_sb[:, 4:8, :], in_=wq_r[:, 4:8, :])
        prefetch_xt(0, 2, 3)
        nc.sync.dma_start(out=wk_sb[:, 0:4, :], in_=wk_r[:, 0:4, :])
        prefetch_xt(0, 3, 4)
        nc.sync.dma_start(out=wk_sb[:, 4:8, :], in_=wk_r[:, 4:8, :])
        wv_r = wv.rearrange("(k p) d -> p k d", p=128)
        nc.sync.dma_start(out=wv_sb[:, 0:4, :], in_=wv_r[:, 0:4, :])
        nc.sync.dma_start(out=r2_sb[:], in_=r2_d[:, :])
        nc.sync.dma_start(out=cos_sb[0:64, :], in_=cos_d[:, :])
        nc.sync.dma_start(out=sin_sb[0:64, :], in_=sin_d[:, :])
        nc.sync.dma_start(out=cos_sb[64:128, :], in_=cos_sb[0:64, :])
        nc.sync.dma_start(out=sin_sb[64:128, :], in_=sin_sb[0:64, :])
        nc.sync.dma_start(out=wv_sb[:, 4:8, :], in_=wv_r[:, 4:8, :])
        nc.sync.dma_start(out=masks_sb[:], in_=masks_d[:, :])
        nc.sync.dma_start(out=wp_sb[:], in_=wp.rearrange("(k p) d -> p k d", p=128))

        onescol = consts.tile([128, NJ, HPC], fmm, tag="onescol")
        nc.vector.memset(onescol[:], 1.0)
        ones64 = consts.tile([1, 64], fmm, tag="ones64")
        nc.vector.memset(ones64[:], 1.0)

        qt_sb = [packs.tile([128, T], fmm, tag=f"qt{p}", name=f"qt{p}") for p in range(2)]
        kt_sb = [packs.tile([128, T], fmm, tag=f"kt{p}", name=f"kt{p}") for p in range(2)]
        ot_sb = [packs.tile([128, T], fmm, tag=f"ot{p}", name=f"ot{p}") for p in range(2)]
        v_sb = packs.tile([128, NJ, HPC * (HD + 1)], fmm, tag="vaug")

        # ones columns of v_aug (fused softmax denominator)
        nc.vector.tensor_copy(
            v_sb[:].rearrange("p j (h c) -> p j h c", h=HPC)[:, :, :, HD:HD + 1],
            onescol[:],
        )

        # --- working pools ---
        tmps = ctx.enter_context(tc.tile_pool(name="tmps", bufs=3))
        pts = ctx.enter_context(tc.tile_pool(name="pts", bufs=16))
        outs = ctx.enter_context(tc.tile_pool(name="outs", bufs=8))
        smalls = ctx.enter_context(tc.tile_pool(name="smalls", bufs=2))

        # 8 PSUM banks: st2(2x2) double-buffered hh-paired score tiles,
        # aux(2) rotated by A's acc/rot/vacc + C's pc, o(2) for oacc pairs.
        psST = ctx.enter_context(tc.tile_pool(name="psST", bufs=2, space="PSUM"))
        psAUX = ctx.enter_context(tc.tile_pool(name="psAUX", bufs=2, space="PSUM"))
        psO = ctx.enter_context(tc.tile_pool(name="psO", bufs=2, space="PSUM"))

        # p-state warmup burst (independent of all real data; runs at t~0).
        # Lives in the st2 pool, whose slots are first needed much later.
        warm_ps = psST.tile([128, 2, 512], f32, tag="st2", name="warm_ps")

        def warm(n):
            for _ in range(n):
                nc.tensor.matmul(warm_ps[:, 0, 0:256], warm_src[:, 0:128],
                                 warm_src[:], start=True, stop=True)
        warm(10)

        def emit_rope(tch, acc, raw, dst, p):
            ts = slice(tch * 512, (tch + 1) * 512)
            rot = psAUX.tile([128, 512], f32, tag="aux", name="rot")
            nc.tensor.matmul(rot[:], r2_sb[:], raw[:], start=True, stop=True)
            # tc_t: both operands bf16 SBUF -> DVE fast mode
            tc_t = tmps.tile([128, 512], bf16, tag="tc", name="tc_t")
            nc.vector.tensor_mul(tc_t[:], raw[:], cos_sb[:, ts])
            # ts_t: reads rot from PSUM (f32) -> 1x
            ts_t = tmps.tile([128, 512], bf16, tag="ts", name="ts_t")
            nc.vector.tensor_mul(ts_t[:], rot[:], sin_sb[:, ts])
            nc.vector.tensor_add(dst[p][:, ts], tc_t[:], ts_t[:])

        def attn_stream(qi, p, oaccs, pending, kj_lo, kj_hi, kj_max):
            """st2 (both head-halves) -> one exp -> (mask) -> O matmuls for
            kj in [kj_lo, kj_hi), software-pipelined so the PE never queues
            behind its own exp."""
            def emit_o(kj, pt2, lo):
                for hh in range(2):
                    hl = 2 * p + hh
                    nc.tensor.matmul(
                        oaccs[hh][0:65, lo:512],
                        v_sb[:, kj, 65 * hl:65 * hl + 65],
                        pt2[:, hh, lo:512],
                        start=(kj == 0), stop=(kj == kj_max - 1),
                    )

            for kj in range(kj_lo, kj_hi):
                r = kj - 4 * qi          # >= 0 on the causal diagonal
                lo = max(r, 0) * 128     # first valid column
                # drain ahead of the new st pair: the queued PV matmuls run
                # while the pair waits for its PSUM slot (exp of pair k-2)
                while len(pending) > 12:
                    emit_o(*pending.pop(0))
                st2 = psST.tile([128, 2, 512], f32, tag="st2", name="st2")
                for hh in range(2):
                    off = 64 * hh
                    nc.tensor.matmul(
                        st2[:, hh, lo:512],
                        kt_sb[p][off:off + 64, kj * 128:(kj + 1) * 128],
                        qt_sb[p][off:off + 64, qi * 512 + lo:(qi + 1) * 512],
                        start=True, stop=True,
                    )
                pt2 = pts.tile([128, 2, 512], fmm, tag="pt", name="pt2")
                nc.scalar.activation(
                    pt2[:, :, lo:512], st2[:, :, lo:512],
                    mybir.ActivationFunctionType.Exp, scale=0.125,
                )
                if r >= 0:
                    for hh in range(2):
                        nc.vector.tensor_mul(
                            pt2[:, hh, lo:lo + 128], pt2[:, hh, lo:lo + 128],
                            masks_sb[:],
                        )
                pending.append((kj, pt2, lo))
            if kj_hi == kj_max:
                while pending:
                    emit_o(*pending.pop(0))

        def attn_norm(qi, p, oaccs):
            qs = slice(qi * 512, (qi + 1) * 512)
            final = qi == NT - 1 and p == 1
            for hh in range(2):
                off = 64 * hh
                recip = smalls.tile([1, 512], fmm, tag="recip", name="recip")
                with nc.allow_low_precision(reason="bf16 rounding of softmax recip"):
                    nc.vector.reciprocal(recip[:], oaccs[hh][64:65, :])
                bc = smalls.tile([64, 512], fmm, tag="bc", name="bc")
                if final:
                    # latency-critical last stream: PE broadcast + ACT copies
                    # (both engines idle here), bf16 operands so the closing
                    # DVE multiply runs in 2x mode.
                    bc_ps = psST.tile([128, 2, 512], f32, tag="st2", name="bc_ps")
                    nc.tensor.matmul(bc_ps[0:64, 0, :], ones64[:], recip[:],
                                     start=True, stop=True)
                    nc.scalar.copy(bc[:], bc_ps[0:64, 0, :])
                    osb = smalls.tile([64, 512], fmm, tag="osb", name="osb")
                    nc.scalar.copy(osb[:], oaccs[hh][0:64, :])
                    nc.vector.tensor_mul(ot_sb[p][off:off + 64, qs], osb[:], bc[:])
                else:
                    nc.gpsimd.partition_broadcast(bc[:], recip[:])
                    nc.vector.tensor_mul(ot_sb[p][off:off + 64, qs], oaccs[hh][0:64, :], bc[:])

        def a_unit_list(tch):
            """A(tch) as a list of emission closures (proj groups, V blocks).
            The rope skew chains across units via `state`."""
            if tch >= NT:
                return []
            state = {"pend": None}

            def start():
                prefetch_xt(tch)
                prefetch_xt(tch + 1)

            def qk_group(w_sb, dst, p, chain_warm=False):
                def emit():
                    xt = xt_tiles[tch]
                    # A(0) runs before B(0): the oacc banks are free, so acc
                    # and vacc chains use them, keeping the 2 aux slots for
                    # rot tiles only (a rot between two accs would make vacc
                    # wait on the sin-DMA-gated rope multiply).
                    if tch == 0:
                        acc = psO.tile([128, 512], f32, tag="oacc", name=f"acc{tch}_{p}")
                    else:
                        acc = psAUX.tile([128, 512], f32, tag="aux", name=f"acc{tch}_{p}")
                    for kc in range(KC):
                        nc.tensor.matmul(
                            acc[:],
                            w_sb[:, kc, 128 * p:128 * (p + 1)],
                            xt[:, kc, :],
                            start=(kc == 0), stop=(kc == KC - 1),
                        )
                        if chain_warm:
                            # keep PE busy while DMA feeds the chain
                            warm({1: 3, 3: 6, 5: 3}.get(kc, 0))
                    raw = tmps.tile([128, 512], fmm, tag="raw", name="raw")
                    nc.vector.tensor_copy(raw[:], acc[:])
                    if state["pend"] is not None:
                        emit_rope(*state["pend"])
                    state["pend"] = (tch, acc, raw, dst, p)
                return emit

            def v_block(jb):
                def emit():
                    xt = xt_tiles[tch]
                    if state["pend"] is not None:
                        emit_rope(*state["pend"])
                        state["pend"] = None
                    jbg = tch * 4 + jb
                    if tch == 0:
                        vacc = psO.tile([128, 256], f32, tag="oacc", name=f"vacc{jbg}")
                    else:
                        vacc = psAUX.tile([128, 256], f32, tag="aux", name=f"vacc{jbg}")
                    for kc in range(KC):
                        nc.tensor.matmul(
                            vacc[:],
                            xt[:, kc, 128 * jb:128 * (jb + 1)],
                            wv_sb[:, kc, :],
                            start=(kc == 0), stop=(kc == KC - 1),
                        )
                    nc.vector.tensor_copy(
                        v_sb[:].rearrange("p j (h c) -> p j h c", h=HPC)[:, jbg, :, 0:HD],
                        vacc[:].rearrange("p (h c) -> p h c", h=HPC),
                    )
                return emit

            units = [start]
            units.append(qk_group(wq_sb, qt_sb, 0, chain_warm=(tch == 0)))
            if tch == 0:
                pass
                # keep the PE busy-streak continuous while DMA feeds the
                # first chains, so the p-state ramp completes early; pull the
                # wv-gated V blocks ahead of the pack-1 chains (p0's stream
                # needs them first anyway)
                units.append(lambda: warm(3))
                units.append(qk_group(wk_sb, kt_sb, 0))
                units.append(lambda: warm(3))
                units.append(v_block(0))
                units.append(v_block(1))
                units.append(qk_group(wq_sb, qt_sb, 1))
                units.append(qk_group(wk_sb, kt_sb, 1))
                units.append(v_block(2))
                units.append(v_block(3))
            else:
                units.append(qk_group(wk_sb, kt_sb, 0))
                units.append(v_block(0))
                units.append(v_block(1))
                units.append(qk_group(wq_sb, qt_sb, 1))
                units.append(qk_group(wk_sb, kt_sb, 1))
                units.append(v_block(2))
                units.append(v_block(3))
            units.append(lambda: xt_tiles.pop(tch, None))
            return units

        def c_unit_list(tch):
            ts = slice(tch * 512, (tch + 1) * 512)
            last = tch == NT - 1
            oc2s = {}

            def c_block(ech):
                def emit():
                    # The trailing C(3) has no B to hide in: alternate its pc
                    # tiles over two pools (4 banks) and spread the PSUM->SBUF
                    # copies over three engines so the pc WAR chain pipelines.
                    if last and ech % 2 == 1:
                        pc = psO.tile([128, 512], f32, tag="oacc", name=f"pc{tch}_{ech}")
                    else:
                        pc = psAUX.tile([128, 512], f32, tag="aux", name=f"pc{tch}_{ech}")
                    for kd in range(2):
                        nc.tensor.matmul(
                            pc[:],
                            wp_sb[:, kd, ech * 128:(ech + 1) * 128],
                            ot_sb[kd][:, ts],
                            start=(kd == 0), stop=(kd == 1),
                        )
                    # pair two ech blocks into one [128, 2, 512] staging tile
                    # -> a single DMA per pair (halves the per-store HWDGE tax)
                    pair = ech // 2
                    if pair not in oc2s:
                        oc2s[pair] = outs.tile([128, 2, 512], bf16, tag="oc", name="oc2")
                    oc2 = oc2s[pair]
                    half = ech % 2
                    # GPSIMD cannot read PSUM: alternate staging copies
                    # between DVE and ACT so neither engine backs up under
                    # the B-stream it fills
                    if ech % 2 == 1:
                        nc.scalar.copy(oc2[:, half, :], pc[:])
                    else:
                        nc.vector.tensor_copy(oc2[:, half, :], pc[:])
                    if half == 1:
                        nc.sync.dma_start(
                            out=yT[2 * pair:2 * pair + 2, tch].rearrange("e p t -> p e t"),
                            in_=oc2[:])
                return emit
            return [c_block(e) for e in range(8)]

        def b_emit(qi, fill_units):
            """Emit B(qi)'s two attention streams, sprinkling `fill_units`
            (A(qi+1) / C(qi-1) closures) between kj iterations so the PE
            always has independent matmul work queued behind exp waits."""
            kj_max = 4 * (qi + 1)
            n_slots = 2 * kj_max + 2
            fill = list(fill_units)
            n_fill = len(fill)
            slot = [0]

            def maybe_fill():
                # spread the n_fill units evenly across the n_slots slots
                k = (n_fill * (slot[0] + 1)) // n_slots - (n_fill * slot[0]) // n_slots
                for _ in range(k):
                    if fill:
                        fill.pop(0)()
                slot[0] += 1

            for p in range(2):
                # qi=3 p=1: take oaccs from the aux pool so this stream's
                # emit_o doesn't wait on p=0's norm to release the psO slots
                # (there is no A/C fill left to hide that wait behind).
                oaccs = [psO.tile([128, 512], f32, tag="oacc", name=f"oacc{qi}_{p}_{j}")
                         for j in range(2)]
                pending = []
                for kj in range(kj_max):
                    attn_stream(qi, p, oaccs, pending, kj, kj + 1, kj_max)
                    maybe_fill()
                attn_norm(qi, p, oaccs)
                maybe_fill()
            while fill:
                fill.pop(0)()

        # software pipeline: A(0); B(0) with A(1) filled in; B(i) with
        # C(i-1)+A(i+1) filled in; C(1) deferred to B(3), which is otherwise
        # fill-starved (no A(4)); trailing C(3).
        for u in a_unit_list(0):
            u()
        b_emit(0, a_unit_list(1))
        b_emit(1, a_unit_list(2))
        b_emit(2, a_unit_list(3))
        b_emit(3, c_unit_list(0) + c_unit_list(1) + c_unit_list(2))
        for u in c_unit_list(3):
            u()

    nc.compile()
    return nc


def get_program():
    global _PROGRAM
    if _PROGRAM is None:
        _PROGRAM = build_program()
    return _PROGRAM


def make_in_maps(x, W_qkv, W_proj):
    x = np.asarray(x, dtype=np.float32)
    W_qkv = np.asarray(W_qkv, dtype=np.float32)
    W_proj = np.asarray(W_proj, dtype=np.float32)
    in_maps = []
    xtr = {}
    for b in range(B):
        xt = x[b].T.reshape(D // 128, 128, T // 512, 512)
        xtr[b] = np.ascontiguousarray(xt.transpose(0, 2, 1, 3)).astype(BF16)
    for core in range(NCORES):
        b, g = divmod(core, 4)
        cs = slice(g * 256, (g + 1) * 256)
        in_maps.append({
            "xT": xtr[b],
            "wq": np.ascontiguousarray(W_qkv[:, 0 * D:1 * D][:, cs]).astype(BF16),
            "wk": np.ascontiguousarray(W_qkv[:, 1 * D:2 * D][:, cs]).astype(BF16),
            "wv": np.ascontiguousarray(W_qkv[:, 2 * D:3 * D][:, cs]).astype(BF16),
            "wp": np.ascontiguousarray(W_proj[cs, :]).astype(BF16),
        })
    return in_maps


def gather_output(results):
    out = np.empty((B, T, D), dtype=np.float32)
    for b in range(B):
        acc = results[4 * b]["yT"].astype(np.float32)
        for g in range(1, 4):
            acc += results[4 * b + g]["yT"].astype(np.float32)
        # (ech, tch, p, t) -> yT (D, T) -> transpose to (T, D)
        yt = acc.transpose(0, 2, 1, 3).reshape(D, T)
        out[b] = yt.T
    return out


def kernel(x, W_qkv, W_proj, key_padding_mask=None, **_ignored):
    # key_padding_mask is all-True per the problem spec (fill: ones) -> no-op.
    from concourse.bass_utils import run_bass_kernel_spmd

    nc = get_program()
    in_maps = make_in_maps(x, W_qkv, W_proj)
    res = run_bass_kernel_spmd(nc, in_maps, list(range(NCORES)))
    return gather_output(res.results)
